# revision 6
# baseline (speedup 1.0000x reference)
"""AGNN (3-layer) Trainium2 kernel.

Strategy (see spec sharding_hint): nodes are partitioned across the 8
NeuronCores by destination (graph/data parallel). Edges are routed to the
core owning the destination node. Per core, destination nodes are grouped
into degree classes (in-degree padded to a multiple of 4, capped at 128);
each 128-slot "block" holds m = 128//K nodes' padded edge lists stacked on
partitions. Per-destination segment softmax and the weighted aggregation
are computed with fixed per-class 0/1 indicator matmuls on the tensor
engine (contraction over partitions), the per-edge cosine logits with the
vector engine, exp on the scalar engine.

The device kernel is a pure streaming+compute NEFF (one compile, one run
per layer). Source-node feature rows for every edge slot are gathered on
the host between layers (the normalized features table is small and the
hardware available here has no fast data-dependent gather primitive — the
extended-ucode DMA gather is absent and indirect DMA runs at ~265ns/row,
measured, which is orders of magnitude off the roofline).
"""

import math
import os
import numpy as np
from contextlib import ExitStack

N_NODES = 100000
D = 32
N_CORES = 8
NPC = N_NODES // N_CORES          # nodes per core
EPS = 1e-12
PAY = D + 1                        # slot payload: xn (32) + norm (1)
CB = 16                            # blocks per compute chunk
SUBRUN = 224                       # max blocks per softmax subrun
_NEFF_CACHE = {}


# ----------------------------------------------------------------------------
# host-side graph preprocessing (layer-invariant)
# ----------------------------------------------------------------------------

class Plan:
    pass


def build_plan(src, dst):
    """src/dst: int64 [E_tot] edge endpoints including self loops."""
    E = src.shape[0]
    owner = dst // NPC

    deg = np.bincount(dst, minlength=N_NODES)
    assert deg.max() <= 128, f"max in-degree {deg.max()} > 128 unsupported"
    K_of_node = 4 * np.ceil(deg / 4).astype(np.int64)
    K_of_node = np.maximum(K_of_node, 4)

    # per-core: sort local nodes by K descending (stable)
    plan = Plan()
    plan.core_nodes = []        # per core: original node ids in sorted order
    plan.core_Ks = []
    for c in range(N_CORES):
        nodes = np.arange(c * NPC, (c + 1) * NPC)
        order = np.argsort(-K_of_node[nodes], kind="stable")
        plan.core_nodes.append(nodes[order])
        plan.core_Ks.append(K_of_node[nodes[order]])

    # class structure equalized across cores
    all_K = sorted(set(int(k) for c in range(N_CORES) for k in plan.core_Ks[c]),
                   reverse=True)
    plan.classes = []           # list of (K, m, nblk)
    for K in all_K:
        m = 128 // K
        nblk = 0
        for c in range(N_CORES):
            nk = int((plan.core_Ks[c] == K).sum())
            nblk = max(nblk, (nk + m - 1) // m)
        plan.classes.append((K, m, nblk))
    plan.NBLK = sum(nblk for _, _, nblk in plan.classes)
    plan.ARRW = plan.NBLK * D

    # per class: column offset in the arrangement tensors
    plan.class_arr_off = []
    off = 0
    for (K, m, nblk) in plan.classes:
        plan.class_arr_off.append(off)
        off += nblk * D
    plan.class_blk_off = []
    off = 0
    for (K, m, nblk) in plan.classes:
        plan.class_blk_off.append(off)
        off += nblk

    # per-core slot->src map [128, NBLK], mask [128, NBLK],
    # node placement map (class arrangement)
    # edges grouped by dst: CSR over sorted nodes
    e_order = np.argsort(dst, kind="stable")
    src_by_dst = src[e_order]
    row_start = np.zeros(N_NODES + 1, dtype=np.int64)
    np.cumsum(deg, out=row_start[1:])

    plan.slot_src = np.zeros((N_CORES, 128, plan.NBLK), dtype=np.int32)
    plan.mask = np.zeros((N_CORES, 128, plan.NBLK), dtype=np.float32)
    # arrangement: for core c, class ci, block b, j -> original node id (or -1)
    plan.arr_node = np.full((N_CORES, 32, plan.NBLK), -1, dtype=np.int64)

    for c in range(N_CORES):
        Ks = plan.core_Ks[c]
        nodes_sorted = plan.core_nodes[c]
        pos = 0
        for ci, (K, m, nblk) in enumerate(plan.classes):
            nk = int((Ks == K).sum())
            cls_nodes = nodes_sorted[pos:pos + nk]
            pos += nk
            b0 = plan.class_blk_off[ci]
            for j_global in range(nk):
                b = j_global // m
                j = j_global % m
                node = cls_nodes[j_global]
                plan.arr_node[c, j, b0 + b] = node
                d0 = deg[node]
                p0 = j * K
                ss = src_by_dst[row_start[node]:row_start[node] + d0]
                plan.slot_src[c, p0:p0 + d0, b0 + b] = ss
                plan.mask[c, p0:p0 + d0, b0 + b] = 1.0
    return plan


def host_normalize(x):
    nrm = np.sqrt((x.astype(np.float64) ** 2).sum(axis=1))
    nrm = np.maximum(nrm, EPS).astype(np.float32)
    xn = (x / nrm[:, None]).astype(np.float32)
    return xn, nrm


def host_layer_inputs(plan, x_full, beta):
    """Build per-core device inputs for one layer from the full node features."""
    xn, nrm = host_normalize(x_full)
    ins = []
    for c in range(N_CORES):
        ss = plan.slot_src[c]                       # [128, NBLK]
        xsl = np.empty((128, plan.NBLK, PAY), dtype=np.float32)
        xsl[:, :, :D] = xn[ss]
        xsl[:, :, D] = nrm[ss]
        xarr = np.zeros((32, plan.ARRW), dtype=np.float32)
        for ci, (K, m, nblk) in enumerate(plan.classes):
            a0 = plan.class_arr_off[ci]
            b0 = plan.class_blk_off[ci]
            nodes = plan.arr_node[c, :m, b0:b0 + nblk]     # [m, nblk]
            valid = nodes >= 0
            xa = np.zeros((m, nblk, D), dtype=np.float32)
            xa[valid] = beta * xn[nodes[valid]]
            xarr[:m, a0:a0 + nblk * D] = xa.reshape(m, nblk * D)
        ins.append({
            "xsl": np.ascontiguousarray(xsl.reshape(128, plan.NBLK * PAY)),
            "xarr": xarr,
            "mask": np.ascontiguousarray(plan.mask[c]),
        })
    return ins


def host_collect_output(plan, oarrs):
    """oarrs: per-core [32, ARRW] aggregation results -> full [N, D]."""
    out = np.zeros((N_NODES, D), dtype=np.float32)
    for c in range(N_CORES):
        oa = oarrs[c]
        for ci, (K, m, nblk) in enumerate(plan.classes):
            a0 = plan.class_arr_off[ci]
            b0 = plan.class_blk_off[ci]
            nodes = plan.arr_node[c, :m, b0:b0 + nblk]     # [m, nblk]
            vals = oa[:m, a0:a0 + nblk * D].reshape(m, nblk, D)
            valid = nodes >= 0
            out[nodes[valid]] = vals[valid]
    return out


# ----------------------------------------------------------------------------
# device kernel
# ----------------------------------------------------------------------------

def host_indicators(plan):
    """Packed per-class indicator matrices (identical for every core)."""
    uniq = []
    seen = set()
    for (K, m, nblk) in plan.classes:
        if (K, m) not in seen:
            seen.add((K, m))
            uniq.append((K, m))
    plan.ind_uniq = uniq
    indk = np.zeros((128, sum(m for _, m in uniq)), dtype=np.float32)
    indkt = np.zeros((32, 128 * len(uniq)), dtype=np.float32)
    plan.ind_off = {}
    off = 0
    for i, (K, m) in enumerate(uniq):
        plan.ind_off[(K, m)] = (off, i)
        p = np.arange(128)
        sel = (p // K) < m
        indk[sel, off + (p // K)[sel]] = 1.0
        indkt[:m, i * 128:(i + 1) * 128] = indk[:, off:off + m].T
        off += m
    plan.indk_w = indk.shape[1]
    plan.n_ind = len(uniq)
    return indk, indkt


def build_nc(plan):
    import concourse.bass as bass
    import concourse.tile as tile
    from concourse import bacc, mybir

    f32 = mybir.dt.float32
    nc = bacc.Bacc("TRN2", target_bir_lowering=False, debug=False)
    xsl_d = nc.declare_dram_parameter("xsl", [128, plan.NBLK * PAY], f32, isOutput=False)
    xarr_d = nc.declare_dram_parameter("xarr", [32, plan.ARRW], f32, isOutput=False)
    mask_d = nc.declare_dram_parameter("mask", [128, plan.NBLK], f32, isOutput=False)
    indk_d = nc.declare_dram_parameter("indk", [128, plan.indk_w], f32, isOutput=False)
    indkt_d = nc.declare_dram_parameter("indkt", [32, 128 * plan.n_ind], f32, isOutput=False)
    oarr_d = nc.declare_dram_parameter("oarr", [32, plan.ARRW], f32, isOutput=True)

    # subrun schedule: (class_idx, blk_off_in_class, nblk_sub)
    subruns = []
    for ci, (K, m, nblk) in enumerate(plan.classes):
        b = 0
        while b < nblk:
            n = min(SUBRUN, nblk - b)
            subruns.append((ci, b, n))
            b += n

    with tile.TileContext(nc) as tc, ExitStack() as ctx:
        const = ctx.enter_context(tc.tile_pool(name="const", bufs=1))
        xpool = ctx.enter_context(tc.tile_pool(name="xsl", bufs=2))
        apool = ctx.enter_context(tc.tile_pool(name="arr", bufs=2))
        wpool = ctx.enter_context(tc.tile_pool(name="work", bufs=2))
        spool = ctx.enter_context(tc.tile_pool(name="small", bufs=2))
        opool = ctx.enter_context(tc.tile_pool(name="outp", bufs=2))
        ps_x = ctx.enter_context(tc.tile_pool(name="psx", bufs=2, space="PSUM"))
        ps_s = ctx.enter_context(tc.tile_pool(name="pss", bufs=2, space="PSUM"))
        ps_a = ctx.enter_context(tc.tile_pool(name="psa", bufs=2, space="PSUM"))

        # resident constants
        mask_sb = const.tile([128, plan.NBLK], f32)
        nc.sync.dma_start(out=mask_sb[:], in_=mask_d[:])
        indk_sb = const.tile([128, plan.indk_w], f32)
        nc.sync.dma_start(out=indk_sb[:], in_=indk_d[:])
        indkt_sb = const.tile([32, 128 * plan.n_ind], f32)
        nc.sync.dma_start(out=indkt_sb[:], in_=indkt_d[:])

        for (ci, bo, R) in subruns:
            K, m, nblk = plan.classes[ci]
            a0 = plan.class_arr_off[ci] + bo * D
            b0 = plan.class_blk_off[ci] + bo
            ioff, iidx = plan.ind_off[(K, m)]
            indk = indk_sb[:, ioff:ioff + m]
            indkt = indkt_sb[:, iidx * 128:(iidx + 1) * 128]

            xs = xpool.tile([128, SUBRUN * PAY], f32, tag="xs")
            nc.sync.dma_start(out=xs[:, :R * PAY],
                              in_=xsl_d[:, b0 * PAY:(b0 + R) * PAY])
            xs3 = xs[:, :R * PAY].rearrange("p (b w) -> p b w", b=R, w=PAY)

            xa = apool.tile([32, SUBRUN * D], f32, tag="xa")
            nc.sync.dma_start(out=xa[:m, :R * D], in_=xarr_d[:m, a0:a0 + R * D])

            alpha = spool.tile([128, SUBRUN], f32, tag="alpha")

            nchunk = (R + CB - 1) // CB
            prods = []
            for q in range(nchunk):
                cb = min(CB, R - q * CB)
                xnd = ps_x.tile([128, CB * D], f32, tag="xnd")
                nc.tensor.matmul(out=xnd[:, :cb * D], lhsT=indkt[:m, :],
                                 rhs=xa[:m, q * CB * D:(q * CB + cb) * D],
                                 start=True, stop=True)
                prod = wpool.tile([128, CB * D], f32, tag="prod")
                nc.vector.tensor_tensor(
                    out=prod[:, :cb * D].rearrange("p (b w) -> p b w", b=cb, w=D),
                    in0=xs3[:, q * CB:q * CB + cb, 0:D],
                    in1=xnd[:, :cb * D].rearrange("p (b w) -> p b w", b=cb, w=D),
                    op=mybir.AluOpType.mult)
                nc.vector.tensor_reduce(
                    out=alpha[:, q * CB:q * CB + cb],
                    in_=prod[:, :cb * D].rearrange("p (b w) -> p b w", b=cb, w=D),
                    axis=mybir.AxisListType.X, op=mybir.AluOpType.add)

            # softmax over segments (global-max-free: |alpha| <= |beta|)
            e = spool.tile([128, SUBRUN], f32, tag="e")
            nc.scalar.activation(e[:, :R], alpha[:, :R],
                                 mybir.ActivationFunctionType.Exp, 0.0, 1.0)
            em = spool.tile([128, SUBRUN], f32, tag="em")
            nc.vector.tensor_tensor(out=em[:, :R], in0=e[:, :R],
                                    in1=mask_sb[:, b0:b0 + R],
                                    op=mybir.AluOpType.mult)
            ss = ps_s.tile([32, SUBRUN], f32, tag="ss")
            nc.tensor.matmul(out=ss[:m, :R], lhsT=indk[:, :m], rhs=em[:, :R],
                             start=True, stop=True)
            ssr = spool.tile([32, SUBRUN], f32, tag="ssr")
            nc.vector.tensor_scalar_add(ssr[:m, :R], ss[:m, :R], 1e-30)
            rs = spool.tile([32, SUBRUN], f32, tag="rs")
            nc.vector.reciprocal(rs[:m, :R], ssr[:m, :R])
            rsrep = ps_s.tile([128, SUBRUN], f32, tag="rsrep")
            nc.tensor.matmul(out=rsrep[:, :R], lhsT=indkt[:m, :], rhs=rs[:m, :R],
                             start=True, stop=True)
            w = spool.tile([128, SUBRUN], f32, tag="w")
            nc.vector.tensor_tensor(out=w[:, :R], in0=em[:, :R], in1=rsrep[:, :R],
                                    op=mybir.AluOpType.mult)
            w2 = spool.tile([128, SUBRUN], f32, tag="w2")
            nc.vector.tensor_tensor(
                out=w2[:, :R], in0=w[:, :R],
                in1=xs3[:, :, D],
                op=mybir.AluOpType.mult)

            for q in range(nchunk):
                cb = min(CB, R - q * CB)
                Y = wpool.tile([128, CB * D], f32, tag="Y")
                w2b = w2[:, q * CB:q * CB + cb, None].to_broadcast([128, cb, D])
                nc.vector.tensor_tensor(
                    out=Y[:, :cb * D].rearrange("p (b w) -> p b w", b=cb, w=D),
                    in0=xs3[:, q * CB:q * CB + cb, 0:D],
                    in1=w2b,
                    op=mybir.AluOpType.mult)
                agg = ps_a.tile([32, CB * D], f32, tag="agg")
                nc.tensor.matmul(out=agg[:m, :cb * D], lhsT=indk[:, :m],
                                 rhs=Y[:, :cb * D], start=True, stop=True)
                oc = opool.tile([32, CB * D], f32, tag="oc")
                nc.scalar.activation(oc[:m, :cb * D], agg[:m, :cb * D],
                                     mybir.ActivationFunctionType.Copy, 0.0, 1.0)
                nc.sync.dma_start(
                    out=oarr_d[:m, a0 + q * CB * D:a0 + (q * CB + cb) * D],
                    in_=oc[:m, :cb * D])

    nc.compile()
    return nc


# ----------------------------------------------------------------------------
# entry point
# ----------------------------------------------------------------------------

def kernel(x, edge_index, beta1, beta2, beta3, _return_debug=False):
    x = np.asarray(x, dtype=np.float32)
    edge_index = np.asarray(edge_index)
    betas = [float(np.asarray(b).reshape(-1)[0]) for b in (beta1, beta2, beta3)]

    loops = np.arange(N_NODES, dtype=edge_index.dtype)
    src = np.concatenate([edge_index[0], loops]).astype(np.int64)
    dst = np.concatenate([edge_index[1], loops]).astype(np.int64)

    plan = build_plan(src, dst)
    indk, indkt = host_indicators(plan)

    from concourse.bass_utils import run_bass_kernel_spmd
    key = (plan.NBLK, tuple(plan.classes))
    if key not in _NEFF_CACHE:
        _NEFF_CACHE[key] = build_nc(plan)
    nc = _NEFF_CACHE[key]

    cur = x
    for li in range(3):
        ins = host_layer_inputs(plan, cur, betas[li])
        for m in ins:
            m["indk"] = indk
            m["indkt"] = indkt
        res = run_bass_kernel_spmd(nc, ins, core_ids=list(range(N_CORES)))
        oarrs = [res.results[c]["oarr"] for c in range(N_CORES)]
        cur = host_collect_output(plan, oarrs)
    return cur


# revision 25
# speedup vs baseline: 34534.4532x; 34534.4532x over previous
"""AGNN (3-layer) Trainium2 kernel.

Strategy (see spec sharding_hint): nodes are partitioned across the 8
NeuronCores by destination (graph/data parallel). Edges are routed to the
core owning the destination node. Per core, destination nodes are grouped
into degree classes (in-degree padded to a multiple of 4, capped at 128);
each 128-slot "block" holds m = 128//K nodes' padded edge lists stacked on
partitions. Per-destination segment softmax and the weighted aggregation
are computed with fixed per-class 0/1 indicator matmuls on the tensor
engine (contraction over partitions), the per-edge cosine logits with the
vector engine, exp on the scalar engine.

The device kernel is a pure streaming+compute NEFF (one compile, one run
per layer). Source-node feature rows for every edge slot are gathered on
the host between layers (the normalized features table is small and the
hardware available here has no fast data-dependent gather primitive — the
extended-ucode DMA gather is absent and indirect DMA runs at ~265ns/row,
measured, which is orders of magnitude off the roofline).
"""

import math
import os
import numpy as np
from contextlib import ExitStack

N_NODES = 100000
D = 32
N_CORES = 8
NPC = N_NODES // N_CORES          # nodes per core
EPS = 1e-12
PAY = D + 1                        # slot payload: xn (32) + norm (1)
CB = 32                            # blocks per compute chunk
SUBRUN = 128                       # max blocks per softmax subrun
Y_ON_GPSIMD = True                 # run the weighted-value pass on the Pool engine
_NEFF_CACHE = {}


# ----------------------------------------------------------------------------
# host-side graph preprocessing (layer-invariant)
# ----------------------------------------------------------------------------

class Plan:
    pass


def build_plan(src, dst):
    """src/dst: int64 [E_tot] edge endpoints including self loops."""
    E = src.shape[0]
    owner = dst // NPC

    deg = np.bincount(dst, minlength=N_NODES)
    assert deg.max() <= 128, f"max in-degree {deg.max()} > 128 unsupported"
    K_of_node = 4 * np.ceil(deg / 4).astype(np.int64)
    K_of_node = np.maximum(K_of_node, 4)

    # per-core: sort local nodes by K descending (stable)
    plan = Plan()
    plan.core_nodes = []        # per core: original node ids in sorted order
    plan.core_Ks = []
    for c in range(N_CORES):
        nodes = np.arange(c * NPC, (c + 1) * NPC)
        order = np.argsort(-K_of_node[nodes], kind="stable")
        plan.core_nodes.append(nodes[order])
        plan.core_Ks.append(K_of_node[nodes[order]])

    # class structure equalized across cores
    all_K = sorted(set(int(k) for c in range(N_CORES) for k in plan.core_Ks[c]),
                   reverse=True)
    plan.classes = []           # list of (K, m, nblk)
    for K in all_K:
        m = 128 // K
        nblk = 0
        for c in range(N_CORES):
            nk = int((plan.core_Ks[c] == K).sum())
            nblk = max(nblk, (nk + m - 1) // m)
        plan.classes.append((K, m, nblk))
    plan.NBLK = sum(nblk for _, _, nblk in plan.classes)
    plan.ARRW = plan.NBLK * D

    # per class: column offset in the arrangement tensors
    plan.class_arr_off = []
    off = 0
    for (K, m, nblk) in plan.classes:
        plan.class_arr_off.append(off)
        off += nblk * D
    plan.class_blk_off = []
    off = 0
    for (K, m, nblk) in plan.classes:
        plan.class_blk_off.append(off)
        off += nblk

    # per-core slot->src map [128, NBLK], mask [128, NBLK],
    # node placement map (class arrangement)
    # edges grouped by dst: CSR over sorted nodes
    e_order = np.argsort(dst, kind="stable")
    src_by_dst = src[e_order]
    row_start = np.zeros(N_NODES + 1, dtype=np.int64)
    np.cumsum(deg, out=row_start[1:])

    plan.slot_src = np.zeros((N_CORES, 128, plan.NBLK), dtype=np.int32)
    plan.mask = np.zeros((N_CORES, 128, plan.NBLK), dtype=np.float32)
    # arrangement: for core c, class ci, block b, j -> original node id (or -1)
    plan.arr_node = np.full((N_CORES, 32, plan.NBLK), -1, dtype=np.int64)

    for c in range(N_CORES):
        Ks = plan.core_Ks[c]
        nodes_sorted = plan.core_nodes[c]
        pos = 0
        for ci, (K, m, nblk) in enumerate(plan.classes):
            nk = int((Ks == K).sum())
            cls_nodes = nodes_sorted[pos:pos + nk]
            pos += nk
            b0 = plan.class_blk_off[ci]
            for j_global in range(nk):
                b = j_global // m
                j = j_global % m
                node = cls_nodes[j_global]
                plan.arr_node[c, j, b0 + b] = node
                d0 = deg[node]
                p0 = j * K
                ss = src_by_dst[row_start[node]:row_start[node] + d0]
                plan.slot_src[c, p0:p0 + d0, b0 + b] = ss
                plan.mask[c, p0:p0 + d0, b0 + b] = 1.0
    return plan


def host_normalize(x):
    nrm = np.sqrt((x.astype(np.float64) ** 2).sum(axis=1))
    nrm = np.maximum(nrm, EPS).astype(np.float32)
    xn = (x / nrm[:, None]).astype(np.float32)
    return xn, nrm


def host_layer_inputs(plan, x_full, beta):
    """Build per-core device inputs for one layer from the full node features."""
    xn, nrm = host_normalize(x_full)
    ins = []
    for c in range(N_CORES):
        ss = plan.slot_src[c]                       # [128, NBLK]
        xsl = np.empty((128, plan.NBLK, PAY), dtype=np.float32)
        xsl[:, :, :D] = xn[ss]
        xsl[:, :, D] = nrm[ss]
        xarr = np.zeros((32, plan.ARRW), dtype=np.float32)
        for ci, (K, m, nblk) in enumerate(plan.classes):
            a0 = plan.class_arr_off[ci]
            b0 = plan.class_blk_off[ci]
            nodes = plan.arr_node[c, :m, b0:b0 + nblk]     # [m, nblk]
            valid = nodes >= 0
            xa = np.zeros((m, nblk, D), dtype=np.float32)
            xa[valid] = beta * xn[nodes[valid]]
            xarr[:m, a0:a0 + nblk * D] = xa.reshape(m, nblk * D)
        ins.append({
            "xsl": np.ascontiguousarray(xsl.reshape(128, plan.NBLK * PAY)),
            "xarr": xarr,
            "mask": np.ascontiguousarray(plan.mask[c]),
        })
    return ins


def host_collect_output(plan, oarrs):
    """oarrs: per-core [32, ARRW] aggregation results -> full [N, D]."""
    out = np.zeros((N_NODES, D), dtype=np.float32)
    for c in range(N_CORES):
        oa = oarrs[c]
        for ci, (K, m, nblk) in enumerate(plan.classes):
            a0 = plan.class_arr_off[ci]
            b0 = plan.class_blk_off[ci]
            nodes = plan.arr_node[c, :m, b0:b0 + nblk]     # [m, nblk]
            vals = oa[:m, a0:a0 + nblk * D].reshape(m, nblk, D)
            valid = nodes >= 0
            out[nodes[valid]] = vals[valid]
    return out


# ----------------------------------------------------------------------------
# device kernel
# ----------------------------------------------------------------------------

def host_indicators(plan):
    """Packed per-class indicator matrices (identical for every core)."""
    uniq = []
    seen = set()
    for (K, m, nblk) in plan.classes:
        if (K, m) not in seen:
            seen.add((K, m))
            uniq.append((K, m))
    plan.ind_uniq = uniq
    indk = np.zeros((128, sum(m for _, m in uniq)), dtype=np.float32)
    indkt = np.zeros((32, 128 * len(uniq)), dtype=np.float32)
    plan.ind_off = {}
    off = 0
    for i, (K, m) in enumerate(uniq):
        plan.ind_off[(K, m)] = (off, i)
        p = np.arange(128)
        sel = (p // K) < m
        indk[sel, off + (p // K)[sel]] = 1.0
        indkt[:m, i * 128:(i + 1) * 128] = indk[:, off:off + m].T
        off += m
    plan.indk_w = indk.shape[1]
    plan.n_ind = len(uniq)
    return indk, indkt


def build_nc(plan):
    import concourse.bass as bass
    import concourse.tile as tile
    from concourse import bacc, mybir

    f32 = mybir.dt.float32
    nc = bacc.Bacc("TRN2", target_bir_lowering=False, debug=False)
    xsl_d = nc.declare_dram_parameter("xsl", [128, plan.NBLK * PAY], f32, isOutput=False)
    xarr_d = nc.declare_dram_parameter("xarr", [32, plan.ARRW], f32, isOutput=False)
    mask_d = nc.declare_dram_parameter("mask", [128, plan.NBLK], f32, isOutput=False)
    indk_d = nc.declare_dram_parameter("indk", [128, plan.indk_w], f32, isOutput=False)
    indkt_d = nc.declare_dram_parameter("indkt", [32, 128 * plan.n_ind], f32, isOutput=False)
    oarr_d = nc.declare_dram_parameter("oarr", [32, plan.ARRW], f32, isOutput=True)

    # subrun schedule: (class_idx, blk_off_in_class, nblk_sub)
    subruns = []
    for ci, (K, m, nblk) in enumerate(plan.classes):
        b = 0
        while b < nblk:
            n = min(SUBRUN, nblk - b)
            subruns.append((ci, b, n))
            b += n

    with tile.TileContext(nc) as tc, ExitStack() as ctx:
        const = ctx.enter_context(tc.tile_pool(name="const", bufs=1))
        xpool = ctx.enter_context(tc.tile_pool(name="xsl", bufs=5))
        apool = ctx.enter_context(tc.tile_pool(name="arr", bufs=2))
        wpool = ctx.enter_context(tc.tile_pool(name="work", bufs=3))
        spool = ctx.enter_context(tc.tile_pool(name="small", bufs=3))
        opool = ctx.enter_context(tc.tile_pool(name="outp", bufs=2))
        ps_x = ctx.enter_context(tc.tile_pool(name="psx", bufs=2, space="PSUM"))
        ps_s = ctx.enter_context(tc.tile_pool(name="pss", bufs=2, space="PSUM"))
        ps_a = ctx.enter_context(tc.tile_pool(name="psa", bufs=2, space="PSUM"))

        # resident constants
        mask_sb = const.tile([128, plan.NBLK], f32)
        nc.sync.dma_start(out=mask_sb[:], in_=mask_d[:])
        indk_sb = const.tile([128, plan.indk_w], f32)
        nc.sync.dma_start(out=indk_sb[:], in_=indk_d[:])
        indkt_sb = const.tile([32, 128 * plan.n_ind], f32)
        nc.sync.dma_start(out=indkt_sb[:], in_=indkt_d[:])

        state = {}

        def ctx_of(si):
            (ci, bo, R) = subruns[si]
            K, m, nblk = plan.classes[ci]
            a0 = plan.class_arr_off[ci] + bo * D
            b0 = plan.class_blk_off[ci] + bo
            ioff, iidx = plan.ind_off[(K, m)]
            return (K, m, a0, b0,
                    indk_sb[:, ioff:ioff + m],
                    indkt_sb[:, iidx * 128:(iidx + 1) * 128], R)

        def emit_A(si):
            K, m, a0, b0, indk, indkt, R = ctx_of(si)
            xs = xpool.tile([128, SUBRUN * PAY], f32, tag="xs")
            nc.sync.dma_start(out=xs[:, :R * PAY],
                              in_=xsl_d[:, b0 * PAY:(b0 + R) * PAY])
            xs3 = xs[:, :R * PAY].rearrange("p (b w) -> p b w", b=R, w=PAY)
            xa = apool.tile([32, SUBRUN * D], f32, tag="xa")
            nc.sync.dma_start(out=xa[:m, :R * D], in_=xarr_d[:m, a0:a0 + R * D])
            alpha = spool.tile([128, SUBRUN], f32, tag="alpha")
            CBA = 16
            for q in range((R + CBA - 1) // CBA):
                cb = min(CBA, R - q * CBA)
                xnd = ps_x.tile([128, CBA * D], f32, tag="xnd")
                for h in range(0, cb * D, 512):
                    hw_ = min(512, cb * D - h)
                    nc.tensor.matmul(out=xnd[:, h:h + hw_], lhsT=indkt[:m, :],
                                     rhs=xa[:m, q * CBA * D + h:q * CBA * D + h + hw_],
                                     start=True, stop=True)
                prod = wpool.tile([128, CBA * D], f32, tag="prod")
                nc.vector.tensor_tensor(
                    out=prod[:, :cb * D].rearrange("p (b w) -> p b w", b=cb, w=D),
                    in0=xs3[:, q * CBA:q * CBA + cb, 0:D],
                    in1=xnd[:, :cb * D].rearrange("p (b w) -> p b w", b=cb, w=D),
                    op=mybir.AluOpType.mult)
                nc.vector.tensor_reduce(
                    out=alpha[:, q * CBA:q * CBA + cb],
                    in_=prod[:, :cb * D].rearrange("p (b w) -> p b w", b=cb, w=D),
                    axis=mybir.AxisListType.X, op=mybir.AluOpType.add)
            state[si] = {"xs3": xs3, "alpha": alpha}

        def emit_B(si):
            K, m, a0, b0, indk, indkt, R = ctx_of(si)
            st = state[si]
            alpha, xs3 = st["alpha"], st["xs3"]
            # softmax over segments (global-max-free: |alpha| <= |beta|)
            e = spool.tile([128, SUBRUN], f32, tag="e")
            nc.scalar.activation(e[:, :R], alpha[:, :R],
                                 mybir.ActivationFunctionType.Exp, 0.0, 1.0)
            em = spool.tile([128, SUBRUN], f32, tag="em")
            nc.vector.tensor_tensor(out=em[:, :R], in0=e[:, :R],
                                    in1=mask_sb[:, b0:b0 + R],
                                    op=mybir.AluOpType.mult)
            ss = ps_s.tile([128, SUBRUN], f32, tag="sm")
            nc.tensor.matmul(out=ss[:m, :R], lhsT=indk[:, :m], rhs=em[:, :R],
                             start=True, stop=True)
            ssr = spool.tile([32, SUBRUN], f32, tag="ssr")
            nc.vector.tensor_scalar_add(ssr[:m, :R], ss[:m, :R], 1e-30)
            rs = spool.tile([32, SUBRUN], f32, tag="rs")
            nc.vector.reciprocal(rs[:m, :R], ssr[:m, :R])
            rsrep = ps_s.tile([128, SUBRUN], f32, tag="sm")
            nc.tensor.matmul(out=rsrep[:, :R], lhsT=indkt[:m, :], rhs=rs[:m, :R],
                             start=True, stop=True)
            w = spool.tile([128, SUBRUN], f32, tag="w")
            nc.vector.tensor_tensor(out=w[:, :R], in0=em[:, :R], in1=rsrep[:, :R],
                                    op=mybir.AluOpType.mult)
            w2 = spool.tile([128, SUBRUN], f32, tag="w2")
            nc.vector.tensor_tensor(
                out=w2[:, :R], in0=w[:, :R],
                in1=xs3[:, :, D],
                op=mybir.AluOpType.mult)
            st["w2"] = w2

        def emit_C(si):
            K, m, a0, b0, indk, indkt, R = ctx_of(si)
            st = state.pop(si)
            xs3, w2 = st["xs3"], st["w2"]
            for q in range((R + CB - 1) // CB):
                cb = min(CB, R - q * CB)
                y_eng = nc.gpsimd if (Y_ON_GPSIMD and q % 2 == 0) else nc.vector
                Y = wpool.tile([128, CB * D], f32, tag="Y")
                w2b = w2[:, q * CB:q * CB + cb, None].to_broadcast([128, cb, D])
                y_eng.tensor_tensor(
                    out=Y[:, :cb * D].rearrange("p (b w) -> p b w", b=cb, w=D),
                    in0=xs3[:, q * CB:q * CB + cb, 0:D],
                    in1=w2b,
                    op=mybir.AluOpType.mult)
                agg = ps_a.tile([32, CB * D], f32, tag="agg")
                for h in range(0, cb * D, 512):
                    hw_ = min(512, cb * D - h)
                    nc.tensor.matmul(out=agg[:m, h:h + hw_], lhsT=indk[:, :m],
                                     rhs=Y[:, h:h + hw_], start=True, stop=True)
                oc = opool.tile([32, CB * D], f32, tag="oc")
                nc.scalar.activation(oc[:m, :cb * D], agg[:m, :cb * D],
                                     mybir.ActivationFunctionType.Copy, 0.0, 1.0)
                nc.sync.dma_start(
                    out=oarr_d[:m, a0 + q * CB * D:a0 + (q * CB + cb) * D],
                    in_=oc[:m, :cb * D])

        n = len(subruns)
        for t in range(n + 4):
            if t < n:
                emit_A(t)
            if 2 <= t < n + 2:
                emit_B(t - 2)
            if t >= 4:
                emit_C(t - 4)

    nc.compile()
    return nc


# ----------------------------------------------------------------------------
# entry point
# ----------------------------------------------------------------------------

def kernel(x, edge_index, beta1, beta2, beta3, _return_debug=False):
    x = np.asarray(x, dtype=np.float32)
    edge_index = np.asarray(edge_index)
    betas = [float(np.asarray(b).reshape(-1)[0]) for b in (beta1, beta2, beta3)]

    loops = np.arange(N_NODES, dtype=edge_index.dtype)
    src = np.concatenate([edge_index[0], loops]).astype(np.int64)
    dst = np.concatenate([edge_index[1], loops]).astype(np.int64)

    plan = build_plan(src, dst)
    indk, indkt = host_indicators(plan)

    from concourse.bass_utils import run_bass_kernel_spmd
    key = (plan.NBLK, tuple(plan.classes))
    if key not in _NEFF_CACHE:
        _NEFF_CACHE[key] = build_nc(plan)
    nc = _NEFF_CACHE[key]

    cur = x
    for li in range(3):
        ins = host_layer_inputs(plan, cur, betas[li])
        for m in ins:
            m["indk"] = indk
            m["indkt"] = indkt
        res = run_bass_kernel_spmd(nc, ins, core_ids=list(range(N_CORES)))
        oarrs = [res.results[c]["oarr"] for c in range(N_CORES)]
        cur = host_collect_output(plan, oarrs)
    return cur
